# revision 1
# baseline (speedup 1.0000x reference)
"""DIEN (GRU -> DIN attention -> AUGRU -> predict head) on 8 TRN2 NeuronCores.

Pure data parallel: batch 2048 -> 8 shards of 256. Weights replicated.
Per-core layout: feature-on-partition [128, batch] for recurrences and
matmuls; batch-on-partition for softmax / hist scaling.

Self-contained: hardcodes all shapes; builds the Bass program lazily and
caches it.
"""
import sys
import numpy as np

sys.path.insert(0, '/opt/trn_rl_repo')

import ml_dtypes
import concourse.bass as bass
import concourse.tile as tile
from concourse import bacc, mybir
from concourse.bass_utils import run_bass_kernel_spmd
from contextlib import ExitStack

BF = mybir.dt.bfloat16
F32 = mybir.dt.float32
AF = mybir.ActivationFunctionType
OP = mybir.AluOpType
AX = mybir.AxisListType

NCORES = 8
B_FULL, T, D, H = 2048, 100, 128, 128
B = B_FULL // NCORES            # 256 per core
BH = 128                        # b-chunk (partition dim for b-layout)
BG = 4                          # b's per attention tile
NT_ATT = B // BG                # 64 attention tiles of [.., BG*T=400]
bf16 = ml_dtypes.bfloat16

_CACHED = {}


def _bcast_row(nc, dst_ap, dram_row_ap):
    """DMA a [1, N] DRAM row broadcast to [parts, N] SBUF."""
    parts = dst_ap.shape[0]
    nc.sync.dma_start(dst_ap, dram_row_ap.broadcast_to([parts] + list(dram_row_ap.shape[1:])))


def build_nc(debug=False, nphases=5):
    nc = bacc.Bacc(None)
    P = lambda n, s, dt=BF: nc.declare_dram_parameter(n, s, dt, isOutput=False)

    xT = P("xT", [T, D, B])                      # GRU x, [t][d][b] bf16
    xbm = P("xbm", [T, B, D])                    # host-masked x, [t][b][d] bf16
    qT = P("qT", [D, B])
    qT32 = P("qT32", [D, B], F32)
    uT = P("uT", [D, B])
    fmask_b = P("fmask_b", [B, T], F32)          # [b][t] 0/1
    len32 = P("len32", [B, 1], F32)
    selT = P("selT", [T, B])                     # one-hot bf16 [t][b]
    eye = P("eye", [128, 128])

    wih = [P(f"wih_{g}", [D, H]) for g in "rzn"]     # lhsT = W_g.T
    whh = [P(f"whh_{g}", [H, H]) for g in "rzn"]
    bihc = P("bihc", [H, 3], F32)
    bhhc = P("bhhc", [H, 3], F32)

    wa_h = [P(f"wa{g}_h", [H, H]) for g in "ruh"]
    wa_x = [P(f"wa{g}_x", [D, H]) for g in "ruh"]
    ba = [P(f"ba_{g}", [H, 1], F32) for g in "ruh"]

    w0k, w0q, w0d, w0p = (P(f"w0{s}", [D, 80]) for s in "kqdp")
    b0 = P("b0", [80, 1], F32)
    w1 = P("w1", [80, 40])
    b1 = P("b1", [40, 1], F32)
    w2 = P("w2", [40, 1])
    b2rep = P("b2rep", [128, 1], F32)

    ph0 = {}
    for blk in ("u", "q", "h", "m", "a"):
        ph0[blk] = (P(f"ph0_{blk}_a", [D, 128]), P(f"ph0_{blk}_b", [D, 72]))
    bph0a = P("bph0a", [128, 1], F32)
    bph0b = P("bph0b", [72, 1], F32)
    ph1a = P("ph1a", [128, 80])
    ph1b = P("ph1b", [72, 80])
    bph1 = P("bph1", [80, 1], F32)
    ph2 = P("ph2", [80, 1])
    bph2 = P("bph2", [1, 1], F32)

    out = nc.declare_dram_parameter("out", [1, B], F32, isOutput=True)
    dbg = {}
    if debug:
        dbg["keys"] = nc.declare_dram_parameter("d_keys", [D, T * B], F32, isOutput=True)
        dbg["scores"] = nc.declare_dram_parameter("d_scores", [NT_ATT, BG * T], F32, isOutput=True)
        dbg["attn"] = nc.declare_dram_parameter("d_attn", [B, T], F32, isOutput=True)
        dbg["pooled"] = nc.declare_dram_parameter("d_pooled", [D, B], F32, isOutput=True)
        dbg["hist"] = nc.declare_dram_parameter("d_hist", [D, B], F32, isOutput=True)
        dbg["attf"] = nc.declare_dram_parameter("d_attf", [D, B], F32, isOutput=True)

    def _body(tc, ctx):
        cp = ctx.enter_context(tc.tile_pool(name="const", bufs=1))
        big = ctx.enter_context(tc.tile_pool(name="big", bufs=1))
        work = ctx.enter_context(tc.tile_pool(name="work", bufs=3))
        gates = ctx.enter_context(tc.tile_pool(name="gates", bufs=3))
        xp = ctx.enter_context(tc.tile_pool(name="xp", bufs=6))
        dramp = ctx.enter_context(tc.tile_pool(name="dram", bufs=1, space="DRAM"))

        scoresDR = dramp.tile([NT_ATT, BG * T], F32)     # row j = att tile j (b-major)
        attnDR = dramp.tile([T, B], BF)
        pooledDR = dramp.tile([D, B], BF)

        def load(p, dt=None):
            nm = f"c_{p.tensor.name if hasattr(p, 'tensor') else p.name}"
            t = cp.tile(list(p.shape), dt or p.dtype, name=nm, tag=nm)
            nc.sync.dma_start(t[:], p[:])
            return t

        # ---------------- constants ----------------
        eye_t = load(eye)
        qT_t = load(qT)
        qT32_t = load(qT32)
        uT_t = load(uT)
        fmask_t = cp.tile([BH, 2, T], F32)
        nc.sync.dma_start(fmask_t[:], fmask_b[:].rearrange("(c b) t -> b c t", c=2))
        len_t = cp.tile([BH, 2], F32)
        nc.sync.dma_start(len_t[:], len32[:].rearrange("(c b) o -> b (c o)", c=2))
        wih_t = [load(w) for w in wih]
        whh_t = [load(w) for w in whh]
        bihc_t = load(bihc)
        bhhc_t = load(bhhc)
        wa_h_t = [load(w) for w in wa_h]
        wa_x_t = [load(w) for w in wa_x]
        ba_t = [load(w) for w in ba]
        w0k_t, w0q_t, w0d_t, w0p_t = load(w0k), load(w0q), load(w0d), load(w0p)
        b0_t, w1_t, b1_t, w2_t, b2_t = load(b0), load(w1), load(b1), load(w2), load(b2rep)
        ph0_t = {k: (load(a), load(b)) for k, (a, b) in ph0.items()}
        bph0a_t, bph0b_t = load(bph0a), load(bph0b)
        ph1a_t, ph1b_t, bph1_t, ph2_t, bph2_t = load(ph1a), load(ph1b), load(bph1), load(ph2), load(bph2)

        # combined gru biases: b_r = bih_r + bhh_r ; b_z likewise
        b_rz = cp.tile([H, 2], F32)
        nc.vector.tensor_add(b_rz[:], bihc_t[:, 0:2], bhhc_t[:, 0:2])
        b_r, b_z = b_rz[:, 0:1], b_rz[:, 1:2]
        b_in, b_hn = bihc_t[:, 2:3], bhhc_t[:, 2:3]

        # folded attention weights: w0k' = w0k + w0d, w0q' = w0q - w0d
        w0kf = cp.tile([D, 80], BF)
        nc.vector.tensor_add(w0kf[:], w0k_t[:], w0d_t[:])
        w0qf = cp.tile([D, 80], BF)
        nc.vector.tensor_sub(w0qf[:], w0q_t[:], w0d_t[:])

        inv_len = cp.tile([BH, 2], F32)
        nc.vector.reciprocal(inv_len[:], len_t[:])

        zeros_bf = cp.tile([128, B], BF)
        nc.vector.memset(zeros_bf[:], 0.0)

        keysT = big.tile([D, T * B], BF, tag="keys")

        # ================ P1: GRU ================
        with tc.tile_pool(name="gru_ps", bufs=2, space="PSUM") as gps:
            h_prev = zeros_bf[:]
            for t in range(T):
                x_t = xp.tile([D, B], BF, tag="x")
                nc.sync.dma_start(x_t[:], xT[t])
                ps_r = gps.tile([H, B], F32, tag="r")
                ps_z = gps.tile([H, B], F32, tag="z")
                ps_in = gps.tile([H, B], F32, tag="in")
                ps_hn = gps.tile([H, B], F32, tag="hn")
                nc.tensor.matmul(ps_r[:], wih_t[0][:], x_t[:], start=True, stop=False)
                nc.tensor.matmul(ps_r[:], whh_t[0][:], h_prev, start=False, stop=True)
                nc.tensor.matmul(ps_z[:], wih_t[1][:], x_t[:], start=True, stop=False)
                nc.tensor.matmul(ps_z[:], whh_t[1][:], h_prev, start=False, stop=True)
                nc.tensor.matmul(ps_in[:], wih_t[2][:], x_t[:], start=True, stop=True)
                nc.tensor.matmul(ps_hn[:], whh_t[2][:], h_prev, start=True, stop=True)

                r = gates.tile([H, B], BF, tag="r")
                nc.scalar.activation(r[:], ps_r[:], AF.Sigmoid, bias=b_r)
                z = gates.tile([H, B], BF, tag="z")
                nc.scalar.activation(z[:], ps_z[:], AF.Sigmoid, bias=b_z)
                # narg = ps_in + (ps_hn + b_hn) * r
                tmp = work.tile([H, B], F32, tag="tmp")
                nc.vector.scalar_tensor_tensor(tmp[:], ps_hn[:], b_hn, r[:], OP.add, OP.mult)
                narg = work.tile([H, B], F32, tag="narg")
                nc.vector.tensor_add(narg[:], ps_in[:], tmp[:])
                n = gates.tile([H, B], BF, tag="n")
                nc.scalar.activation(n[:], narg[:], AF.Tanh, bias=b_in)
                # h' = n + z*(h - n)
                d = work.tile([H, B], BF, tag="d")
                nc.vector.tensor_sub(d[:], h_prev, n[:])
                zd = work.tile([H, B], BF, tag="zd")
                nc.vector.tensor_mul(zd[:], z[:], d[:])
                h_new = keysT[:, t * B:(t + 1) * B]
                nc.vector.tensor_add(h_new, n[:], zd[:])
                h_prev = h_new

        if debug:
            for j in range(25):
                seg = slice(j * 1024, (j + 1) * 1024)
                tmpd = work.tile([D, 1024], F32, tag="dbgk")
                nc.vector.tensor_copy(tmpd[:], keysT[:, seg])
                nc.sync.dma_start(dbg["keys"][:, seg], tmpd[:])

        # ================ P2: attention MLP + hist ================
        if nphases < 2:
            stub = cp.tile([1, B], F32)
            nc.vector.tensor_copy(stub[:], keysT[0:1, 0:B])
            nc.sync.dma_start(out[:], stub[:])
            return
        ptBIG = big.tile([D, T * B], BF, tag="big2")
        hist_b = [cp.tile([BH, D], BF, name=f"histb{c}") for c in range(2)]
        kv = keysT[:].rearrange("p (t b) -> p t b", t=T)
        pv = ptBIG[:].rearrange("p (t b) -> p t b", t=T)

        with tc.tile_pool(name="att_ps", bufs=2, space="PSUM") as aps, \
             tc.tile_pool(name="hist_ps", bufs=1, space="PSUM") as hps, \
             tc.tile_pool(name="attw", bufs=3) as aw:
            # pT = q * keys (t-major contiguous tiles of 2 t-steps)
            qbc = qT_t[:][:, None, :].broadcast_to([D, 2, B])
            for j in range(T // 2):
                ks = kv[:, 2 * j:2 * j + 2, :]
                ps = pv[:, 2 * j:2 * j + 2, :]
                nc.vector.tensor_mul(ps, ks, qbc)
            # hist: sum over t of host-masked xbm -> [b, d] psum, 2 chunks
            hist_ps = [hps.tile([BH, D], F32, tag=f"h{c}", name=f"histps{c}") for c in range(2)]
            for t in range(T):
                for c in range(2):
                    xt = xp.tile([BH, D], BF, tag=f"xb{c}")
                    nc.sync.dma_start(xt[:], xbm[t, c * BH:(c + 1) * BH, :])
                    nc.tensor.matmul(hist_ps[c][:], eye_t[:], xt[:],
                                     start=(t == 0), stop=(t == T - 1))
            for c in range(2):
                nc.vector.tensor_scalar_mul(hist_b[c][:], hist_ps[c][:], inv_len[:, c:c + 1])

            # attention MLP over b-major tiles
            for j in range(NT_ATT):
                bs = slice(j * BG, (j + 1) * BG)
                k_j = kv[:, :, bs].transpose([0, 2, 1])          # [D, BG, T]
                p_j = pv[:, :, bs].transpose([0, 2, 1])
                q_j = qT_t[:, bs, None].broadcast_to([D, BG, T])
                ps1 = aps.tile([80, BG * T], F32, tag="a1")
                o1 = ps1[:].rearrange("p (b t) -> p b t", b=BG)
                nc.tensor.matmul(o1, w0kf[:], k_j, start=True, stop=False)
                nc.tensor.matmul(o1, w0qf[:], q_j, start=False, stop=False)
                nc.tensor.matmul(o1, w0p_t[:], p_j, start=False, stop=True)
                a1 = aw.tile([80, BG * T], BF, tag="a1s")
                nc.scalar.activation(a1[:], ps1[:], AF.Relu, bias=b0_t[:])
                ps2 = aps.tile([40, BG * T], F32, tag="a2")
                nc.tensor.matmul(ps2[:], w1_t[:], a1[:], start=True, stop=True)
                a2 = aw.tile([40, BG * T], BF, tag="a2s")
                nc.scalar.activation(a2[:], ps2[:], AF.Relu, bias=b1_t[:])
                ps3 = aps.tile([1, BG * T], F32, tag="a3")
                nc.tensor.matmul(ps3[:], w2_t[:], a2[:], start=True, stop=True)
                s3row = aw.tile([1, BG * T], F32, tag="s3row")
                nc.vector.tensor_copy(s3row[:], ps3[:])
                nc.sync.dma_start(scoresDR[j], s3row[:])

        if debug:
            nc.sync.dma_start(dbg["scores"][:], scoresDR[:])

        if nphases < 3:
            stub = cp.tile([1, B], F32)
            nc.sync.dma_start(stub[:], scoresDR[0, None, 0:B])
            nc.sync.dma_start(out[:], stub[:])
            return
        # ================ P3: softmax + pooled + hist transpose ================
        attn_bf = cp.tile([BH, 2 * T], BF)
        attnT_sb = cp.tile([T, B], BF)
        histT32 = cp.tile([D, B], F32)
        scv = scoresDR[:].rearrange("j (b t) -> (j b) t", b=BG)     # [256, 100]
        with tc.tile_pool(name="sm_ps", bufs=2, space="PSUM") as sps, \
             tc.tile_pool(name="smw", bufs=2) as smw:
            for c in range(2):
                sc = smw.tile([BH, T], F32, tag="sc")
                nc.sync.dma_start(sc[:], scv[c * BH:(c + 1) * BH, :])
                E = smw.tile([BH, T], F32, tag="E")
                nc.scalar.activation(E[:], sc[:], AF.Exp, bias=b2_t[:])
                nc.vector.tensor_scalar_max(E[:], E[:], 1.0)
                nc.vector.tensor_mul(E[:], E[:], fmask_t[:, c, :])
                den = smw.tile([BH, 1], F32, tag="den")
                nc.vector.tensor_reduce(den[:], E[:], AX.X, OP.add)
                rec = smw.tile([BH, 1], F32, tag="rec")
                nc.vector.reciprocal(rec[:], den[:])
                nc.vector.tensor_scalar_mul(attn_bf[:, c * T:(c + 1) * T], E[:], rec[:])
                if debug:
                    af = smw.tile([BH, T], F32, tag="af32")
                    nc.vector.tensor_copy(af[:], attn_bf[:, c * T:(c + 1) * T])
                    nc.sync.dma_start(dbg["attn"][c * BH:(c + 1) * BH, :], af[:])
                pst = sps.tile([T, BH], BF, tag="tr")
                nc.tensor.transpose(pst[:], attn_bf[:, c * T:(c + 1) * T], eye_t[:])
                nc.vector.tensor_copy(attnT_sb[:, c * BH:(c + 1) * BH], pst[:])
                psh = sps.tile([D, BH], BF, tag="trh")
                nc.tensor.transpose(psh[:], hist_b[c][:], eye_t[:])
                nc.vector.tensor_copy(histT32[:, c * BH:(c + 1) * BH], psh[:])
        nc.sync.dma_start(attnDR[:], attnT_sb[:])

        # broadcast attn rows -> abig; P = keys * attn_bc; reduce over t
        abig = big.tile([D, T * B], BF, tag="big2")   # reuses ptBIG slot
        for t in range(T):
            _bcast_row(nc, abig[:, t * B:(t + 1) * B], attnDR[t:t + 1, :])
        for j in range(T * B // 512):
            seg = slice(j * 512, (j + 1) * 512)
            nc.vector.tensor_mul(abig[:, seg], keysT[:, seg], abig[:, seg])
        pooledT = cp.tile([D, B], F32)
        av = abig[:].rearrange("p (t b) -> p t b", t=T)
        nc.vector.tensor_reduce(pooledT[:], av.transpose([0, 2, 1]), AX.X, OP.add)
        pooled_bf = cp.tile([D, B], BF)
        nc.vector.tensor_copy(pooled_bf[:], pooledT[:])
        nc.sync.dma_start(pooledDR[:], pooled_bf[:])
        if debug:
            nc.sync.dma_start(dbg["pooled"][:], pooledT[:])
            nc.sync.dma_start(dbg["hist"][:], histT32[:])

        if nphases < 4:
            stub = cp.tile([1, B], F32)
            nc.vector.tensor_copy(stub[:], pooledT[0:1, :])
            nc.sync.dma_start(out[:], stub[:])
            return
        # ================ P4: AUGRU ================
        attf_acc = cp.tile([D, B], F32)
        nc.gpsimd.memset(attf_acc[:], 0.0)
        abc_p = ctx.enter_context(tc.tile_pool(name="abc", bufs=6))
        with tc.tile_pool(name="aug_ps", bufs=2, space="PSUM") as ups:
            h_prev = zeros_bf[:]
            for t in range(T):
                k_t = keysT[:, t * B:(t + 1) * B]
                abc = abc_p.tile([128, B], BF, tag="abc")
                _bcast_row(nc, abc[:], pooledDR[t:t + 1, :])
                selbc = abc_p.tile([128, B], BF, tag="selbc")
                _bcast_row(nc, selbc[:], selT[t:t + 1, :])

                ps_r = ups.tile([H, B], F32, tag="r")
                ps_u = ups.tile([H, B], F32, tag="u")
                ps_h = ups.tile([H, B], F32, tag="hh")
                nc.tensor.matmul(ps_r[:], wa_x_t[0][:], k_t, start=True, stop=False)
                nc.tensor.matmul(ps_r[:], wa_h_t[0][:], h_prev, start=False, stop=True)
                nc.tensor.matmul(ps_u[:], wa_x_t[1][:], k_t, start=True, stop=False)
                nc.tensor.matmul(ps_u[:], wa_h_t[1][:], h_prev, start=False, stop=True)

                r = gates.tile([H, B], BF, tag="ar")
                nc.scalar.activation(r[:], ps_r[:], AF.Sigmoid, bias=ba_t[0][:])
                u = gates.tile([H, B], BF, tag="au")
                nc.scalar.activation(u[:], ps_u[:], AF.Sigmoid, bias=ba_t[1][:])
                rh = gates.tile([H, B], BF, tag="rh")
                nc.vector.tensor_mul(rh[:], r[:], h_prev)
                nc.tensor.matmul(ps_h[:], wa_x_t[2][:], k_t, start=True, stop=False)
                nc.tensor.matmul(ps_h[:], wa_h_t[2][:], rh[:], start=False, stop=True)
                hh = gates.tile([H, B], BF, tag="hh")
                nc.scalar.activation(hh[:], ps_h[:], AF.Tanh, bias=ba_t[2][:])

                up = gates.tile([H, B], BF, tag="up")
                nc.vector.tensor_mul(up[:], abc[:], u[:])
                dd = work.tile([H, B], BF, tag="add")
                nc.vector.tensor_sub(dd[:], hh[:], h_prev)
                ud = work.tile([H, B], BF, tag="aud")
                nc.vector.tensor_mul(ud[:], up[:], dd[:])
                h_new_t = gates.tile([H, B], BF, tag="ah")
                nc.vector.tensor_add(h_new_t[:], h_prev, ud[:])
                # attf += h_new * selbc  (gpsimd, off the critical path)
                sp = work.tile([H, B], BF, tag="sp")
                nc.gpsimd.tensor_mul(sp[:], h_new_t[:], selbc[:])
                nc.gpsimd.tensor_add(attf_acc[:], attf_acc[:], sp[:])
                h_prev = h_new_t[:]

        if nphases < 5:
            stub = cp.tile([1, B], F32)
            nc.vector.tensor_copy(stub[:], attf_acc[0:1, :])
            nc.sync.dma_start(out[:], stub[:])
            return
        # ================ P5: predict head ================
        attf_bf = cp.tile([D, B], BF)
        nc.vector.tensor_copy(attf_bf[:], attf_acc[:])
        if debug:
            nc.sync.dma_start(dbg["attf"][:], attf_acc[:])
        m2 = cp.tile([D, B], F32)
        nc.vector.tensor_mul(m2[:], qT32_t[:], histT32[:])
        m2_bf = cp.tile([D, B], BF)
        nc.vector.tensor_copy(m2_bf[:], m2[:])
        hist_bf = cp.tile([D, B], BF)
        nc.vector.tensor_copy(hist_bf[:], histT32[:])

        comb = [uT_t[:], qT_t[:], hist_bf[:], m2_bf[:], attf_bf[:]]
        with tc.tile_pool(name="ph_ps", bufs=2, space="PSUM") as pps, \
             tc.tile_pool(name="phw", bufs=2) as pw:
            s1a_ps = pps.tile([128, B], F32, tag="s1a")
            s1b_ps = pps.tile([72, B], F32, tag="s1b")
            for i, blk in enumerate(("u", "q", "h", "m", "a")):
                nc.tensor.matmul(s1a_ps[:], ph0_t[blk][0][:], comb[i],
                                 start=(i == 0), stop=(i == 4))
                nc.tensor.matmul(s1b_ps[:], ph0_t[blk][1][:], comb[i],
                                 start=(i == 0), stop=(i == 4))
            s1a = pw.tile([128, B], BF, tag="s1a")
            nc.scalar.activation(s1a[:], s1a_ps[:], AF.Sigmoid, bias=bph0a_t[:])
            s1b = pw.tile([72, B], BF, tag="s1b")
            nc.scalar.activation(s1b[:], s1b_ps[:], AF.Sigmoid, bias=bph0b_t[:])
            s2_ps = pps.tile([80, B], F32, tag="s2")
            nc.tensor.matmul(s2_ps[:], ph1a_t[:], s1a[:], start=True, stop=False)
            nc.tensor.matmul(s2_ps[:], ph1b_t[:], s1b[:], start=False, stop=True)
            s2 = pw.tile([80, B], BF, tag="s2s")
            nc.scalar.activation(s2[:], s2_ps[:], AF.Sigmoid, bias=bph1_t[:])
            s3_ps = pps.tile([1, B], F32, tag="s3")
            nc.tensor.matmul(s3_ps[:], ph2_t[:], s2[:], start=True, stop=True)
            s3 = pw.tile([1, B], F32, tag="s3s")
            nc.scalar.activation(s3[:], s3_ps[:], AF.Sigmoid, bias=bph2_t[0:1, :])
            nc.sync.dma_start(out[:], s3[:])

    with tile.TileContext(nc) as tc, ExitStack() as ctx:
        _body(tc, ctx)
    return _finish(nc)


def _finish(nc):
    if not nc.is_finalized():
        nc.finalize()
    return nc


def _prep_in_maps(inputs):
    f = np.float32
    x = np.asarray(inputs["item_historical_embedding"], f)
    q = np.asarray(inputs["item_embedding"], f)
    u = np.asarray(inputs["user_embedding"], f)
    mask = np.asarray(inputs["mask"])
    lens = np.asarray(inputs["sequential_length"])

    W = {}
    gih = np.asarray(inputs["gru_Wih"], f)     # (3H, D)
    ghh = np.asarray(inputs["gru_Whh"], f)
    for i, g in enumerate("rzn"):
        W[f"wih_{g}"] = np.ascontiguousarray(gih[i * H:(i + 1) * H, :].T).astype(bf16)
        W[f"whh_{g}"] = np.ascontiguousarray(ghh[i * H:(i + 1) * H, :].T).astype(bf16)
    W["bihc"] = np.ascontiguousarray(np.asarray(inputs["gru_bih"], f).reshape(3, H).T)
    W["bhhc"] = np.ascontiguousarray(np.asarray(inputs["gru_bhh"], f).reshape(3, H).T)
    for g, wn, bn in (("r", "aug_Wr", "aug_br"), ("u", "aug_Wu", "aug_bu"),
                      ("h", "aug_Wh", "aug_bh")):
        wa = np.asarray(inputs[wn], f)                                # (H, D+H)
        W[f"wa{g}_h"] = np.ascontiguousarray(wa[:, :H].T).astype(bf16)
        W[f"wa{g}_x"] = np.ascontiguousarray(wa[:, H:].T).astype(bf16)
        W[f"ba_{g}"] = np.asarray(inputs[bn], f).reshape(H, 1)
    a0 = np.asarray(inputs["att_W0"], f)                              # (80, 512)
    for i, s in enumerate("kqdp"):
        W[f"w0{s}"] = np.ascontiguousarray(a0[:, i * D:(i + 1) * D].T).astype(bf16)
    W["b0"] = np.asarray(inputs["att_b0"], f).reshape(80, 1)
    W["w1"] = np.ascontiguousarray(np.asarray(inputs["att_W1"], f).T).astype(bf16)
    W["b1"] = np.asarray(inputs["att_b1"], f).reshape(40, 1)
    W["w2"] = np.ascontiguousarray(np.asarray(inputs["att_W2"], f).T).astype(bf16)
    W["b2rep"] = np.full((128, 1), float(np.asarray(inputs["att_b2"], f).reshape(-1)[0]), f)
    p0 = np.asarray(inputs["ph_W0"], f)                               # (200, 640)
    for i, blk in enumerate(("u", "q", "h", "m", "a")):
        blkW = p0[:, i * D:(i + 1) * D]                               # (200, 128)
        W[f"ph0_{blk}_a"] = np.ascontiguousarray(blkW[:128, :].T).astype(bf16)
        W[f"ph0_{blk}_b"] = np.ascontiguousarray(blkW[128:, :].T).astype(bf16)
    bp0 = np.asarray(inputs["ph_b0"], f)
    W["bph0a"] = bp0[:128].reshape(128, 1)
    W["bph0b"] = bp0[128:].reshape(72, 1)
    p1 = np.asarray(inputs["ph_W1"], f)                               # (80, 200)
    W["ph1a"] = np.ascontiguousarray(p1[:, :128].T).astype(bf16)
    W["ph1b"] = np.ascontiguousarray(p1[:, 128:].T).astype(bf16)
    W["bph1"] = np.asarray(inputs["ph_b1"], f).reshape(80, 1)
    W["ph2"] = np.ascontiguousarray(np.asarray(inputs["ph_W2"], f).T).astype(bf16)
    W["bph2"] = np.asarray(inputs["ph_b2"], f).reshape(1, 1)
    W["eye"] = np.eye(128).astype(bf16)

    in_maps = []
    for s in range(NCORES):
        sl = slice(s * B, (s + 1) * B)
        xs = x[sl]                       # (B, T, D)
        ms = mask[sl]                    # (B, T) int32
        m = dict(W)
        m["xT"] = np.ascontiguousarray(xs.transpose(1, 2, 0)).astype(bf16)   # [T, D, B]
        xm = xs * ms[:, :, None]
        m["xbm"] = np.ascontiguousarray(xm.transpose(1, 0, 2)).astype(bf16)  # [T, B, D]
        m["qT"] = np.ascontiguousarray(q[sl].T).astype(bf16)
        m["qT32"] = np.ascontiguousarray(q[sl].T)
        m["uT"] = np.ascontiguousarray(u[sl].T).astype(bf16)
        m["fmask_b"] = np.ascontiguousarray(ms).astype(f)
        m["len32"] = lens[sl].astype(f).reshape(B, 1)
        sel = np.zeros((T, B), f)
        sel[np.asarray(lens[sl], np.int64) - 1, np.arange(B)] = 1.0
        m["selT"] = sel.astype(bf16)
        in_maps.append(m)
    return in_maps


def get_nc(debug=False, nphases=5):
    key = ("nc", debug, nphases)
    if key not in _CACHED:
        _CACHED[key] = build_nc(debug=debug, nphases=nphases)
    return _CACHED[key]


def run_on_hw(inputs, debug=False):
    nc = get_nc(debug=debug)
    in_maps = _prep_in_maps(inputs)
    return run_bass_kernel_spmd(nc, in_maps, list(range(NCORES)))


def kernel(**inputs) -> np.ndarray:
    r = run_on_hw(inputs, debug=False)
    outs = [r.results[i]["out"].reshape(B) for i in range(NCORES)]
    return np.concatenate(outs).astype(np.float32)



# revision 2
# speedup vs baseline: 3.0265x; 3.0265x over previous
"""DIEN (GRU -> DIN attention -> AUGRU -> predict head) on 8 TRN2 NeuronCores.

Pure data parallel: batch 2048 -> 8 shards of 256. Weights replicated.
Per-core layout: feature-on-partition [128, batch] for recurrences and
matmuls; batch-on-partition for softmax.

Wire-traffic-lean v2: x is sent once, host-masked, in [T, D, B] layout
(mask only affects t >= len, and nothing past len-1 reaches the output),
so hist is just an on-device running sum of x_t scaled by 1/len. The
duplicate [T, B, D] copy and the f32 q copy of the baseline are gone.

Self-contained: hardcodes all shapes; builds the Bass program lazily and
caches it.
"""
import sys
import numpy as np

sys.path.insert(0, '/opt/trn_rl_repo')

import ml_dtypes
import concourse.bass as bass
import concourse.tile as tile
from concourse import bacc, mybir
from concourse.bass_utils import run_bass_kernel_spmd
from contextlib import ExitStack

BF = mybir.dt.bfloat16
F32 = mybir.dt.float32
XDT = mybir.dt.float8e4          # wire dtype for the big x tensor
XNP = ml_dtypes.float8_e4m3
AF = mybir.ActivationFunctionType
OP = mybir.AluOpType
AX = mybir.AxisListType

NCORES = 8
B_FULL, T, D, H = 2048, 100, 128, 128
B = B_FULL // NCORES            # 256 per core
BH = 128                        # b-chunk (partition dim for b-layout)
BG = 4                          # b's per attention tile
NT_ATT = B // BG                # 64 attention tiles of [.., BG*T=400]
bf16 = ml_dtypes.bfloat16

_CACHED = {}


def _bcast_row(nc, dst_ap, dram_row_ap):
    """DMA a [1, N] DRAM row broadcast to [parts, N] SBUF."""
    parts = dst_ap.shape[0]
    nc.sync.dma_start(dst_ap, dram_row_ap.broadcast_to([parts] + list(dram_row_ap.shape[1:])))


def build_nc(debug=False, nphases=5):
    nc = bacc.Bacc(None)
    P = lambda n, s, dt=BF: nc.declare_dram_parameter(n, s, dt, isOutput=False)

    xT = P("xT", [T, D, B], XDT)                 # host-masked GRU x, [t][d][b]
    qT = P("qT", [D, B])
    uT = P("uT", [D, B])
    fmask_b = P("fmask_b", [B, T])               # [b][t] 0/1 bf16
    invlen_row = P("invlen_row", [1, B], F32)    # 1/len per batch col
    selT = P("selT", [T, B])                     # one-hot bf16 [t][b]
    eye = P("eye", [128, 128])

    wih = [P(f"wih_{g}", [D, H]) for g in "rzn"]     # lhsT = W_g.T
    whh = [P(f"whh_{g}", [H, H]) for g in "rzn"]
    bihc = P("bihc", [H, 3], F32)
    bhhc = P("bhhc", [H, 3], F32)

    wa_h = [P(f"wa{g}_h", [H, H]) for g in "ruh"]
    wa_x = [P(f"wa{g}_x", [D, H]) for g in "ruh"]
    ba = [P(f"ba_{g}", [H, 1], F32) for g in "ruh"]

    w0kf, w0qf, w0p = (P(f"w0{s}", [D, 80]) for s in ("kf", "qf", "p"))
    b0 = P("b0", [80, 1], F32)
    w1 = P("w1", [80, 40])
    b1 = P("b1", [40, 1], F32)
    w2 = P("w2", [40, 1])
    b2rep = P("b2rep", [128, 1], F32)

    ph0 = {}
    for blk in ("u", "q", "h", "m", "a"):
        ph0[blk] = (P(f"ph0_{blk}_a", [D, 128]), P(f"ph0_{blk}_b", [D, 72]))
    bph0a = P("bph0a", [128, 1], F32)
    bph0b = P("bph0b", [72, 1], F32)
    ph1a = P("ph1a", [128, 80])
    ph1b = P("ph1b", [72, 80])
    bph1 = P("bph1", [80, 1], F32)
    ph2 = P("ph2", [80, 1])
    bph2 = P("bph2", [1, 1], F32)

    out = nc.declare_dram_parameter("out", [1, B], F32, isOutput=True)
    dbg = {}
    if debug:
        dbg["keys"] = nc.declare_dram_parameter("d_keys", [D, T * B], F32, isOutput=True)
        dbg["scores"] = nc.declare_dram_parameter("d_scores", [NT_ATT, BG * T], F32, isOutput=True)
        dbg["attn"] = nc.declare_dram_parameter("d_attn", [B, T], F32, isOutput=True)
        dbg["pooled"] = nc.declare_dram_parameter("d_pooled", [D, B], F32, isOutput=True)
        dbg["hist"] = nc.declare_dram_parameter("d_hist", [D, B], F32, isOutput=True)
        dbg["attf"] = nc.declare_dram_parameter("d_attf", [D, B], F32, isOutput=True)

    def _body(tc, ctx):
        cp = ctx.enter_context(tc.tile_pool(name="const", bufs=1))
        big = ctx.enter_context(tc.tile_pool(name="big", bufs=1))
        work = ctx.enter_context(tc.tile_pool(name="work", bufs=3))
        gates = ctx.enter_context(tc.tile_pool(name="gates", bufs=3))
        xp = ctx.enter_context(tc.tile_pool(name="xp", bufs=6))
        dramp = ctx.enter_context(tc.tile_pool(name="dram", bufs=1, space="DRAM"))

        scoresDR = dramp.tile([NT_ATT, BG * T], F32)     # row j = att tile j (b-major)
        attnDR = dramp.tile([T, B], BF)
        pooledDR = dramp.tile([D, B], BF)

        def load(p, dt=None):
            nm = f"c_{p.tensor.name if hasattr(p, 'tensor') else p.name}"
            t = cp.tile(list(p.shape), dt or p.dtype, name=nm, tag=nm)
            nc.sync.dma_start(t[:], p[:])
            return t

        # ---------------- constants ----------------
        eye_t = load(eye)
        qT_t = load(qT)
        uT_t = load(uT)
        fmask_t = cp.tile([BH, 2, T], BF)
        nc.sync.dma_start(fmask_t[:], fmask_b[:].rearrange("(c b) t -> b c t", c=2))
        wih_t = [load(w) for w in wih]
        whh_t = [load(w) for w in whh]
        bihc_t = load(bihc)
        bhhc_t = load(bhhc)
        wa_h_t = [load(w) for w in wa_h]
        wa_x_t = [load(w) for w in wa_x]
        ba_t = [load(w) for w in ba]
        w0kf_t, w0qf_t, w0p_t = load(w0kf), load(w0qf), load(w0p)
        b0_t, w1_t, b1_t, w2_t, b2_t = load(b0), load(w1), load(b1), load(w2), load(b2rep)
        ph0_t = {k: (load(a), load(b)) for k, (a, b) in ph0.items()}
        bph0a_t, bph0b_t = load(bph0a), load(bph0b)
        ph1a_t, ph1b_t, bph1_t, ph2_t, bph2_t = load(ph1a), load(ph1b), load(bph1), load(ph2), load(bph2)

        invlen_bc = cp.tile([D, B], F32)
        _bcast_row(nc, invlen_bc[:], invlen_row[:])

        # combined gru biases: b_r = bih_r + bhh_r ; b_z likewise
        b_rz = cp.tile([H, 2], F32)
        nc.vector.tensor_add(b_rz[:], bihc_t[:, 0:2], bhhc_t[:, 0:2])
        b_r, b_z = b_rz[:, 0:1], b_rz[:, 1:2]
        b_in, b_hn = bihc_t[:, 2:3], bhhc_t[:, 2:3]

        zeros_bf = cp.tile([128, B], BF)
        nc.vector.memset(zeros_bf[:], 0.0)
        hist_acc = cp.tile([D, B], F32)
        nc.gpsimd.memset(hist_acc[:], 0.0)

        keysT = big.tile([D, T * B], BF, tag="keys")

        # ================ P1: GRU (+ hist accumulation on gpsimd) ================
        with tc.tile_pool(name="gru_ps", bufs=2, space="PSUM") as gps:
            h_prev = zeros_bf[:]
            for t in range(T):
                x_raw = xp.tile([D, B], XDT, tag="xr")
                nc.sync.dma_start(x_raw[:], xT[t])
                x_t = xp.tile([D, B], BF, tag="x")
                nc.scalar.activation(x_t[:], x_raw[:], AF.Copy)
                nc.gpsimd.tensor_add(hist_acc[:], hist_acc[:], x_t[:])
                ps_r = gps.tile([H, B], F32, tag="r")
                ps_z = gps.tile([H, B], F32, tag="z")
                ps_in = gps.tile([H, B], F32, tag="in")
                ps_hn = gps.tile([H, B], F32, tag="hn")
                nc.tensor.matmul(ps_r[:], wih_t[0][:], x_t[:], start=True, stop=False)
                nc.tensor.matmul(ps_r[:], whh_t[0][:], h_prev, start=False, stop=True)
                nc.tensor.matmul(ps_z[:], wih_t[1][:], x_t[:], start=True, stop=False)
                nc.tensor.matmul(ps_z[:], whh_t[1][:], h_prev, start=False, stop=True)
                nc.tensor.matmul(ps_in[:], wih_t[2][:], x_t[:], start=True, stop=True)
                nc.tensor.matmul(ps_hn[:], whh_t[2][:], h_prev, start=True, stop=True)

                r = gates.tile([H, B], BF, tag="r")
                nc.scalar.activation(r[:], ps_r[:], AF.Sigmoid, bias=b_r)
                z = gates.tile([H, B], BF, tag="z")
                nc.scalar.activation(z[:], ps_z[:], AF.Sigmoid, bias=b_z)
                # narg = ps_in + (ps_hn + b_hn) * r
                tmp = work.tile([H, B], F32, tag="tmp")
                nc.vector.scalar_tensor_tensor(tmp[:], ps_hn[:], b_hn, r[:], OP.add, OP.mult)
                narg = work.tile([H, B], F32, tag="narg")
                nc.vector.tensor_add(narg[:], ps_in[:], tmp[:])
                n = gates.tile([H, B], BF, tag="n")
                nc.scalar.activation(n[:], narg[:], AF.Tanh, bias=b_in)
                # h' = n + z*(h - n)
                d = work.tile([H, B], BF, tag="d")
                nc.vector.tensor_sub(d[:], h_prev, n[:])
                zd = work.tile([H, B], BF, tag="zd")
                nc.vector.tensor_mul(zd[:], z[:], d[:])
                h_new = keysT[:, t * B:(t + 1) * B]
                nc.vector.tensor_add(h_new, n[:], zd[:])
                h_prev = h_new

        histT32 = cp.tile([D, B], F32)
        nc.vector.tensor_mul(histT32[:], hist_acc[:], invlen_bc[:])

        if debug:
            for j in range(25):
                seg = slice(j * 1024, (j + 1) * 1024)
                tmpd = work.tile([D, 1024], F32, tag="dbgk")
                nc.vector.tensor_copy(tmpd[:], keysT[:, seg])
                nc.sync.dma_start(dbg["keys"][:, seg], tmpd[:])

        # ================ P2: attention MLP ================
        if nphases < 2:
            stub = cp.tile([1, B], F32)
            nc.vector.tensor_copy(stub[:], keysT[0:1, 0:B])
            nc.sync.dma_start(out[:], stub[:])
            return
        ptBIG = big.tile([D, T * B], BF, tag="big2")
        kv = keysT[:].rearrange("p (t b) -> p t b", t=T)
        pv = ptBIG[:].rearrange("p (t b) -> p t b", t=T)

        with tc.tile_pool(name="att_ps", bufs=2, space="PSUM") as aps, \
             tc.tile_pool(name="attw", bufs=3) as aw:
            # pT = q * keys (t-major contiguous tiles of 2 t-steps)
            qbc = qT_t[:][:, None, :].broadcast_to([D, 2, B])
            for j in range(T // 2):
                ks = kv[:, 2 * j:2 * j + 2, :]
                ps = pv[:, 2 * j:2 * j + 2, :]
                nc.vector.tensor_mul(ps, ks, qbc)

            # attention MLP over b-major tiles
            for j in range(NT_ATT):
                bs = slice(j * BG, (j + 1) * BG)
                k_j = kv[:, :, bs].transpose([0, 2, 1])          # [D, BG, T]
                p_j = pv[:, :, bs].transpose([0, 2, 1])
                q_j = qT_t[:, bs, None].broadcast_to([D, BG, T])
                ps1 = aps.tile([80, BG * T], F32, tag="a1")
                o1 = ps1[:].rearrange("p (b t) -> p b t", b=BG)
                nc.tensor.matmul(o1, w0kf_t[:], k_j, start=True, stop=False)
                nc.tensor.matmul(o1, w0qf_t[:], q_j, start=False, stop=False)
                nc.tensor.matmul(o1, w0p_t[:], p_j, start=False, stop=True)
                a1 = aw.tile([80, BG * T], BF, tag="a1s")
                nc.scalar.activation(a1[:], ps1[:], AF.Relu, bias=b0_t[:])
                ps2 = aps.tile([40, BG * T], F32, tag="a2")
                nc.tensor.matmul(ps2[:], w1_t[:], a1[:], start=True, stop=True)
                a2 = aw.tile([40, BG * T], BF, tag="a2s")
                nc.scalar.activation(a2[:], ps2[:], AF.Relu, bias=b1_t[:])
                ps3 = aps.tile([1, BG * T], F32, tag="a3")
                nc.tensor.matmul(ps3[:], w2_t[:], a2[:], start=True, stop=True)
                s3row = aw.tile([1, BG * T], F32, tag="s3row")
                nc.vector.tensor_copy(s3row[:], ps3[:])
                nc.sync.dma_start(scoresDR[j], s3row[:])

        if debug:
            nc.sync.dma_start(dbg["scores"][:], scoresDR[:])

        if nphases < 3:
            stub = cp.tile([1, B], F32)
            nc.sync.dma_start(stub[:], scoresDR[0, None, 0:B])
            nc.sync.dma_start(out[:], stub[:])
            return
        # ================ P3: softmax + pooled ================
        attn_bf = cp.tile([BH, 2 * T], BF)
        attnT_sb = cp.tile([T, B], BF)
        scv = scoresDR[:].rearrange("j (b t) -> (j b) t", b=BG)     # [256, 100]
        with tc.tile_pool(name="sm_ps", bufs=2, space="PSUM") as sps, \
             tc.tile_pool(name="smw", bufs=2) as smw:
            for c in range(2):
                sc = smw.tile([BH, T], F32, tag="sc")
                nc.sync.dma_start(sc[:], scv[c * BH:(c + 1) * BH, :])
                E = smw.tile([BH, T], F32, tag="E")
                nc.scalar.activation(E[:], sc[:], AF.Exp, bias=b2_t[:])
                nc.vector.tensor_scalar_max(E[:], E[:], 1.0)
                nc.vector.tensor_mul(E[:], E[:], fmask_t[:, c, :])
                den = smw.tile([BH, 1], F32, tag="den")
                nc.vector.tensor_reduce(den[:], E[:], AX.X, OP.add)
                rec = smw.tile([BH, 1], F32, tag="rec")
                nc.vector.reciprocal(rec[:], den[:])
                nc.vector.tensor_scalar_mul(attn_bf[:, c * T:(c + 1) * T], E[:], rec[:])
                if debug:
                    af = smw.tile([BH, T], F32, tag="af32")
                    nc.vector.tensor_copy(af[:], attn_bf[:, c * T:(c + 1) * T])
                    nc.sync.dma_start(dbg["attn"][c * BH:(c + 1) * BH, :], af[:])
                pst = sps.tile([T, BH], BF, tag="tr")
                nc.tensor.transpose(pst[:], attn_bf[:, c * T:(c + 1) * T], eye_t[:])
                nc.vector.tensor_copy(attnT_sb[:, c * BH:(c + 1) * BH], pst[:])
        nc.sync.dma_start(attnDR[:], attnT_sb[:])

        # broadcast attn rows -> abig; P = keys * attn_bc; reduce over t
        abig = big.tile([D, T * B], BF, tag="big2")   # reuses ptBIG slot
        for t in range(T):
            _bcast_row(nc, abig[:, t * B:(t + 1) * B], attnDR[t:t + 1, :])
        for j in range(T * B // 512):
            seg = slice(j * 512, (j + 1) * 512)
            nc.vector.tensor_mul(abig[:, seg], keysT[:, seg], abig[:, seg])
        pooledT = cp.tile([D, B], F32)
        av = abig[:].rearrange("p (t b) -> p t b", t=T)
        nc.vector.tensor_reduce(pooledT[:], av.transpose([0, 2, 1]), AX.X, OP.add)
        pooled_bf = cp.tile([D, B], BF)
        nc.vector.tensor_copy(pooled_bf[:], pooledT[:])
        nc.sync.dma_start(pooledDR[:], pooled_bf[:])
        if debug:
            nc.sync.dma_start(dbg["pooled"][:], pooledT[:])
            nc.sync.dma_start(dbg["hist"][:], histT32[:])

        if nphases < 4:
            stub = cp.tile([1, B], F32)
            nc.vector.tensor_copy(stub[:], pooledT[0:1, :])
            nc.sync.dma_start(out[:], stub[:])
            return
        # ================ P4: AUGRU ================
        attf_acc = cp.tile([D, B], F32)
        nc.gpsimd.memset(attf_acc[:], 0.0)
        abc_p = ctx.enter_context(tc.tile_pool(name="abc", bufs=6))
        with tc.tile_pool(name="aug_ps", bufs=2, space="PSUM") as ups:
            h_prev = zeros_bf[:]
            for t in range(T):
                k_t = keysT[:, t * B:(t + 1) * B]
                abc = abc_p.tile([128, B], BF, tag="abc")
                _bcast_row(nc, abc[:], pooledDR[t:t + 1, :])
                selbc = abc_p.tile([128, B], BF, tag="selbc")
                _bcast_row(nc, selbc[:], selT[t:t + 1, :])

                ps_r = ups.tile([H, B], F32, tag="r")
                ps_u = ups.tile([H, B], F32, tag="u")
                ps_h = ups.tile([H, B], F32, tag="hh")
                nc.tensor.matmul(ps_r[:], wa_x_t[0][:], k_t, start=True, stop=False)
                nc.tensor.matmul(ps_r[:], wa_h_t[0][:], h_prev, start=False, stop=True)
                nc.tensor.matmul(ps_u[:], wa_x_t[1][:], k_t, start=True, stop=False)
                nc.tensor.matmul(ps_u[:], wa_h_t[1][:], h_prev, start=False, stop=True)

                r = gates.tile([H, B], BF, tag="ar")
                nc.scalar.activation(r[:], ps_r[:], AF.Sigmoid, bias=ba_t[0][:])
                u = gates.tile([H, B], BF, tag="au")
                nc.scalar.activation(u[:], ps_u[:], AF.Sigmoid, bias=ba_t[1][:])
                rh = gates.tile([H, B], BF, tag="rh")
                nc.vector.tensor_mul(rh[:], r[:], h_prev)
                nc.tensor.matmul(ps_h[:], wa_x_t[2][:], k_t, start=True, stop=False)
                nc.tensor.matmul(ps_h[:], wa_h_t[2][:], rh[:], start=False, stop=True)
                hh = gates.tile([H, B], BF, tag="hh")
                nc.scalar.activation(hh[:], ps_h[:], AF.Tanh, bias=ba_t[2][:])

                up = gates.tile([H, B], BF, tag="up")
                nc.vector.tensor_mul(up[:], abc[:], u[:])
                dd = work.tile([H, B], BF, tag="add")
                nc.vector.tensor_sub(dd[:], hh[:], h_prev)
                ud = work.tile([H, B], BF, tag="aud")
                nc.vector.tensor_mul(ud[:], up[:], dd[:])
                h_new_t = gates.tile([H, B], BF, tag="ah")
                nc.vector.tensor_add(h_new_t[:], h_prev, ud[:])
                # attf += h_new * selbc  (gpsimd, off the critical path)
                sp = work.tile([H, B], BF, tag="sp")
                nc.gpsimd.tensor_mul(sp[:], h_new_t[:], selbc[:])
                nc.gpsimd.tensor_add(attf_acc[:], attf_acc[:], sp[:])
                h_prev = h_new_t[:]

        if nphases < 5:
            stub = cp.tile([1, B], F32)
            nc.vector.tensor_copy(stub[:], attf_acc[0:1, :])
            nc.sync.dma_start(out[:], stub[:])
            return
        # ================ P5: predict head ================
        attf_bf = cp.tile([D, B], BF)
        nc.vector.tensor_copy(attf_bf[:], attf_acc[:])
        if debug:
            nc.sync.dma_start(dbg["attf"][:], attf_acc[:])
        m2_bf = cp.tile([D, B], BF)
        nc.vector.tensor_mul(m2_bf[:], qT_t[:], histT32[:])
        hist_bf = cp.tile([D, B], BF)
        nc.vector.tensor_copy(hist_bf[:], histT32[:])

        comb = [uT_t[:], qT_t[:], hist_bf[:], m2_bf[:], attf_bf[:]]
        with tc.tile_pool(name="ph_ps", bufs=2, space="PSUM") as pps, \
             tc.tile_pool(name="phw", bufs=2) as pw:
            s1a_ps = pps.tile([128, B], F32, tag="s1a")
            s1b_ps = pps.tile([72, B], F32, tag="s1b")
            for i, blk in enumerate(("u", "q", "h", "m", "a")):
                nc.tensor.matmul(s1a_ps[:], ph0_t[blk][0][:], comb[i],
                                 start=(i == 0), stop=(i == 4))
                nc.tensor.matmul(s1b_ps[:], ph0_t[blk][1][:], comb[i],
                                 start=(i == 0), stop=(i == 4))
            s1a = pw.tile([128, B], BF, tag="s1a")
            nc.scalar.activation(s1a[:], s1a_ps[:], AF.Sigmoid, bias=bph0a_t[:])
            s1b = pw.tile([72, B], BF, tag="s1b")
            nc.scalar.activation(s1b[:], s1b_ps[:], AF.Sigmoid, bias=bph0b_t[:])
            s2_ps = pps.tile([80, B], F32, tag="s2")
            nc.tensor.matmul(s2_ps[:], ph1a_t[:], s1a[:], start=True, stop=False)
            nc.tensor.matmul(s2_ps[:], ph1b_t[:], s1b[:], start=False, stop=True)
            s2 = pw.tile([80, B], BF, tag="s2s")
            nc.scalar.activation(s2[:], s2_ps[:], AF.Sigmoid, bias=bph1_t[:])
            s3_ps = pps.tile([1, B], F32, tag="s3")
            nc.tensor.matmul(s3_ps[:], ph2_t[:], s2[:], start=True, stop=True)
            s3 = pw.tile([1, B], F32, tag="s3s")
            nc.scalar.activation(s3[:], s3_ps[:], AF.Sigmoid, bias=bph2_t[0:1, :])
            nc.sync.dma_start(out[:], s3[:])

    with tile.TileContext(nc) as tc, ExitStack() as ctx:
        _body(tc, ctx)
    return _finish(nc)


def _finish(nc):
    if not nc.is_finalized():
        nc.finalize()
    return nc


def _prep_in_maps(inputs):
    f = np.float32
    x = np.asarray(inputs["item_historical_embedding"], f)
    q = np.asarray(inputs["item_embedding"], f)
    u = np.asarray(inputs["user_embedding"], f)
    mask = np.asarray(inputs["mask"])
    lens = np.asarray(inputs["sequential_length"])

    W = {}
    gih = np.asarray(inputs["gru_Wih"], f)     # (3H, D)
    ghh = np.asarray(inputs["gru_Whh"], f)
    for i, g in enumerate("rzn"):
        W[f"wih_{g}"] = np.ascontiguousarray(gih[i * H:(i + 1) * H, :].T).astype(bf16)
        W[f"whh_{g}"] = np.ascontiguousarray(ghh[i * H:(i + 1) * H, :].T).astype(bf16)
    W["bihc"] = np.ascontiguousarray(np.asarray(inputs["gru_bih"], f).reshape(3, H).T)
    W["bhhc"] = np.ascontiguousarray(np.asarray(inputs["gru_bhh"], f).reshape(3, H).T)
    for g, wn, bn in (("r", "aug_Wr", "aug_br"), ("u", "aug_Wu", "aug_bu"),
                      ("h", "aug_Wh", "aug_bh")):
        wa = np.asarray(inputs[wn], f)                                # (H, D+H)
        W[f"wa{g}_h"] = np.ascontiguousarray(wa[:, :H].T).astype(bf16)
        W[f"wa{g}_x"] = np.ascontiguousarray(wa[:, H:].T).astype(bf16)
        W[f"ba_{g}"] = np.asarray(inputs[bn], f).reshape(H, 1)
    a0 = np.asarray(inputs["att_W0"], f)                              # (80, 512)
    w0k, w0q, w0d, w0p = (np.ascontiguousarray(a0[:, i * D:(i + 1) * D].T)
                          for i in range(4))
    W["w0kf"] = (w0k + w0d).astype(bf16)
    W["w0qf"] = (w0q - w0d).astype(bf16)
    W["w0p"] = w0p.astype(bf16)
    W["b0"] = np.asarray(inputs["att_b0"], f).reshape(80, 1)
    W["w1"] = np.ascontiguousarray(np.asarray(inputs["att_W1"], f).T).astype(bf16)
    W["b1"] = np.asarray(inputs["att_b1"], f).reshape(40, 1)
    W["w2"] = np.ascontiguousarray(np.asarray(inputs["att_W2"], f).T).astype(bf16)
    W["b2rep"] = np.full((128, 1), float(np.asarray(inputs["att_b2"], f).reshape(-1)[0]), f)
    p0 = np.asarray(inputs["ph_W0"], f)                               # (200, 640)
    for i, blk in enumerate(("u", "q", "h", "m", "a")):
        blkW = p0[:, i * D:(i + 1) * D]                               # (200, 128)
        W[f"ph0_{blk}_a"] = np.ascontiguousarray(blkW[:128, :].T).astype(bf16)
        W[f"ph0_{blk}_b"] = np.ascontiguousarray(blkW[128:, :].T).astype(bf16)
    bp0 = np.asarray(inputs["ph_b0"], f)
    W["bph0a"] = bp0[:128].reshape(128, 1)
    W["bph0b"] = bp0[128:].reshape(72, 1)
    p1 = np.asarray(inputs["ph_W1"], f)                               # (80, 200)
    W["ph1a"] = np.ascontiguousarray(p1[:, :128].T).astype(bf16)
    W["ph1b"] = np.ascontiguousarray(p1[:, 128:].T).astype(bf16)
    W["bph1"] = np.asarray(inputs["ph_b1"], f).reshape(80, 1)
    W["ph2"] = np.ascontiguousarray(np.asarray(inputs["ph_W2"], f).T).astype(bf16)
    W["bph2"] = np.asarray(inputs["ph_b2"], f).reshape(1, 1)
    W["eye"] = np.eye(128).astype(bf16)

    in_maps = []
    for s in range(NCORES):
        sl = slice(s * B, (s + 1) * B)
        xs = x[sl]                       # (B, T, D)
        ms = mask[sl]                    # (B, T) int32
        m = dict(W)
        xm = xs * ms[:, :, None]         # host-masked: t >= len rows are 0
        m["xT"] = np.ascontiguousarray(xm.transpose(1, 2, 0)).astype(XNP)    # [T, D, B]
        m["qT"] = np.ascontiguousarray(q[sl].T).astype(bf16)
        m["uT"] = np.ascontiguousarray(u[sl].T).astype(bf16)
        m["fmask_b"] = np.ascontiguousarray(ms).astype(bf16)
        m["invlen_row"] = (1.0 / lens[sl].astype(f)).reshape(1, B)
        sel = np.zeros((T, B), f)
        sel[np.asarray(lens[sl], np.int64) - 1, np.arange(B)] = 1.0
        m["selT"] = sel.astype(bf16)
        in_maps.append(m)
    return in_maps


def get_nc(debug=False, nphases=5):
    key = ("nc", debug, nphases)
    if key not in _CACHED:
        _CACHED[key] = build_nc(debug=debug, nphases=nphases)
    return _CACHED[key]


def run_on_hw(inputs, debug=False):
    nc = get_nc(debug=debug)
    in_maps = _prep_in_maps(inputs)
    return run_bass_kernel_spmd(nc, in_maps, list(range(NCORES)))


def kernel(**inputs) -> np.ndarray:
    r = run_on_hw(inputs, debug=False)
    outs = [r.results[i]["out"].reshape(B) for i in range(NCORES)]
    return np.concatenate(outs).astype(np.float32)


# revision 3
# speedup vs baseline: 3.7027x; 1.2234x over previous
"""DIEN (GRU -> DIN attention -> AUGRU -> predict head) on 8 TRN2 NeuronCores.

Pure data parallel: batch 2048 -> 8 shards of 256. Weights replicated.
Per-core layout: feature-on-partition [128, batch] for recurrences and
matmuls; batch-on-partition for softmax.

Wire-traffic-lean: the runtime ships every DRAM parameter as its own
transfer with ~15ms fixed cost, so all small tensors (weights, biases,
q/u embeddings, masks) are packed into two flat blobs (bf16 + f32) that
the kernel slices with DMA. The big x tensor is sent once, host-masked
(mask only affects t >= len, and nothing past len-1 reaches the output)
as fp8-e4m3 in [T, D, B] layout; hist is an on-device running sum.

Self-contained: hardcodes all shapes; builds the Bass program lazily and
caches it.
"""
import sys
import numpy as np

sys.path.insert(0, '/opt/trn_rl_repo')

import ml_dtypes
import concourse.bass as bass
import concourse.tile as tile
from concourse import bacc, mybir
from concourse.bass_utils import run_bass_kernel_spmd
from contextlib import ExitStack

BF = mybir.dt.bfloat16
F32 = mybir.dt.float32
XDT = mybir.dt.float8e4          # wire dtype for the big x tensor
XNP = ml_dtypes.float8_e4m3
AF = mybir.ActivationFunctionType
OP = mybir.AluOpType
AX = mybir.AxisListType

NCORES = 8
B_FULL, T, D, H = 2048, 100, 128, 128
B = B_FULL // NCORES            # 256 per core
BH = 128                        # b-chunk (partition dim for b-layout)
BG = 4                          # b's per attention tile
NT_ATT = B // BG                # 64 attention tiles of [.., BG*T=400]
bf16 = ml_dtypes.bfloat16

# ---- packed-blob layouts: (name, shape); order defines blob offsets ----
PACK_BF = (
    [("qT", (D, B)), ("uT", (D, B)), ("fmask3", (BH, 2, T)),
     ("selT", (T, B)), ("eye", (128, 128))]
    + [(f"wih_{g}", (D, H)) for g in "rzn"]
    + [(f"whh_{g}", (H, H)) for g in "rzn"]
    + [(f"wa{g}_h", (H, H)) for g in "ruh"]
    + [(f"wa{g}_x", (D, H)) for g in "ruh"]
    + [("w0kf", (D, 80)), ("w0qf", (D, 80)), ("w0p", (D, 80)),
       ("w1", (80, 40)), ("w2", (40, 1))]
    + [(f"ph0_{blk}_{h}", (D, 128 if h == "a" else 72))
       for blk in "uqhma" for h in "ab"]
    + [("ph1a", (128, 80)), ("ph1b", (72, 80)), ("ph2", (80, 1))]
)
PACK_F32 = [
    ("bihc", (H, 3)), ("bhhc", (H, 3)),
    ("ba_r", (H, 1)), ("ba_u", (H, 1)), ("ba_h", (H, 1)),
    ("b0", (80, 1)), ("b1", (40, 1)), ("b2rep", (128, 1)),
    ("bph0a", (128, 1)), ("bph0b", (72, 1)), ("bph1", (80, 1)), ("bph2", (1, 1)),
    ("invlen_row", (1, B)),
]


def _offsets(pack):
    off, table = 0, {}
    for name, shape in pack:
        n = int(np.prod(shape))
        table[name] = (off, shape)
        off += n
    return table, off


OFF_BF, N_BF = _offsets(PACK_BF)
OFF_F32, N_F32 = _offsets(PACK_F32)

_CACHED = {}


def _bcast_row(nc, dst_ap, dram_row_ap):
    """DMA a [1, N] DRAM row broadcast to [parts, N] SBUF."""
    parts = dst_ap.shape[0]
    nc.sync.dma_start(dst_ap, dram_row_ap.broadcast_to([parts] + list(dram_row_ap.shape[1:])))


def build_nc(debug=False, nphases=5):
    nc = bacc.Bacc(None)

    xT = nc.declare_dram_parameter("xT", [T, D, B], XDT, isOutput=False)
    wbf = nc.declare_dram_parameter("wbf", [1, N_BF], BF, isOutput=False)
    wf32 = nc.declare_dram_parameter("wf32", [1, N_F32], F32, isOutput=False)
    out = nc.declare_dram_parameter("out", [1, B], F32, isOutput=True)

    def bview(name):
        blob, table = (wbf, OFF_BF) if name in OFF_BF else (wf32, OFF_F32)
        off, shape = table[name]
        n = int(np.prod(shape))
        v = blob[:, off:off + n]
        if len(shape) == 2:
            return v.rearrange("o (a b) -> (o a) b", a=shape[0])
        if len(shape) == 3:
            return v.rearrange("o (a b c) -> (o a) b c", a=shape[0], b=shape[1])
        return v

    dbg = {}
    if debug:
        dbg["keys"] = nc.declare_dram_parameter("d_keys", [D, T * B], F32, isOutput=True)
        dbg["scores"] = nc.declare_dram_parameter("d_scores", [NT_ATT, BG * T], F32, isOutput=True)
        dbg["attn"] = nc.declare_dram_parameter("d_attn", [B, T], F32, isOutput=True)
        dbg["pooled"] = nc.declare_dram_parameter("d_pooled", [D, B], F32, isOutput=True)
        dbg["hist"] = nc.declare_dram_parameter("d_hist", [D, B], F32, isOutput=True)
        dbg["attf"] = nc.declare_dram_parameter("d_attf", [D, B], F32, isOutput=True)

    def _body(tc, ctx):
        cp = ctx.enter_context(tc.tile_pool(name="const", bufs=1))
        big = ctx.enter_context(tc.tile_pool(name="big", bufs=1))
        work = ctx.enter_context(tc.tile_pool(name="work", bufs=3))
        gates = ctx.enter_context(tc.tile_pool(name="gates", bufs=3))
        xp = ctx.enter_context(tc.tile_pool(name="xp", bufs=6))
        dramp = ctx.enter_context(tc.tile_pool(name="dram", bufs=1, space="DRAM"))

        scoresDR = dramp.tile([NT_ATT, BG * T], F32)     # row j = att tile j (b-major)
        attnDR = dramp.tile([T, B], BF)
        pooledDR = dramp.tile([D, B], BF)

        def load(name, dt=None):
            table = OFF_BF if name in OFF_BF else OFF_F32
            shape = table[name][1]
            t = cp.tile(list(shape), dt or (BF if name in OFF_BF else F32),
                        name=f"c_{name}", tag=f"c_{name}")
            nc.sync.dma_start(t[:], bview(name))
            return t

        # ---------------- constants ----------------
        eye_t = load("eye")
        qT_t = load("qT")
        uT_t = load("uT")
        fmask_t = load("fmask3")
        wih_t = [load(f"wih_{g}") for g in "rzn"]
        whh_t = [load(f"whh_{g}") for g in "rzn"]
        bihc_t = load("bihc")
        bhhc_t = load("bhhc")
        wa_h_t = [load(f"wa{g}_h") for g in "ruh"]
        wa_x_t = [load(f"wa{g}_x") for g in "ruh"]
        ba_t = [load(f"ba_{g}") for g in "ruh"]
        w0kf_t, w0qf_t, w0p_t = load("w0kf"), load("w0qf"), load("w0p")
        b0_t, w1_t, b1_t, w2_t, b2_t = load("b0"), load("w1"), load("b1"), load("w2"), load("b2rep")
        ph0_t = {blk: (load(f"ph0_{blk}_a"), load(f"ph0_{blk}_b")) for blk in "uqhma"}
        bph0a_t, bph0b_t = load("bph0a"), load("bph0b")
        ph1a_t, ph1b_t, bph1_t, ph2_t, bph2_t = (load("ph1a"), load("ph1b"),
                                                 load("bph1"), load("ph2"), load("bph2"))

        invlen_bc = cp.tile([D, B], F32)
        _bcast_row(nc, invlen_bc[:], bview("invlen_row"))

        # combined gru biases: b_r = bih_r + bhh_r ; b_z likewise
        b_rz = cp.tile([H, 2], F32)
        nc.vector.tensor_add(b_rz[:], bihc_t[:, 0:2], bhhc_t[:, 0:2])
        b_r, b_z = b_rz[:, 0:1], b_rz[:, 1:2]
        b_in, b_hn = bihc_t[:, 2:3], bhhc_t[:, 2:3]

        zeros_bf = cp.tile([128, B], BF)
        nc.vector.memset(zeros_bf[:], 0.0)
        hist_acc = cp.tile([D, B], F32)
        nc.gpsimd.memset(hist_acc[:], 0.0)

        keysT = big.tile([D, T * B], BF, tag="keys")

        # ================ P1: GRU (+ hist accumulation on gpsimd) ================
        with tc.tile_pool(name="gru_ps", bufs=2, space="PSUM") as gps:
            h_prev = zeros_bf[:]
            for t in range(T):
                x_raw = xp.tile([D, B], XDT, tag="xr")
                nc.sync.dma_start(x_raw[:], xT[t])
                x_t = xp.tile([D, B], BF, tag="x")
                nc.scalar.activation(x_t[:], x_raw[:], AF.Copy)
                nc.gpsimd.tensor_add(hist_acc[:], hist_acc[:], x_t[:])
                ps_r = gps.tile([H, B], F32, tag="r")
                ps_z = gps.tile([H, B], F32, tag="z")
                ps_in = gps.tile([H, B], F32, tag="in")
                ps_hn = gps.tile([H, B], F32, tag="hn")
                nc.tensor.matmul(ps_r[:], wih_t[0][:], x_t[:], start=True, stop=False)
                nc.tensor.matmul(ps_r[:], whh_t[0][:], h_prev, start=False, stop=True)
                nc.tensor.matmul(ps_z[:], wih_t[1][:], x_t[:], start=True, stop=False)
                nc.tensor.matmul(ps_z[:], whh_t[1][:], h_prev, start=False, stop=True)
                nc.tensor.matmul(ps_in[:], wih_t[2][:], x_t[:], start=True, stop=True)
                nc.tensor.matmul(ps_hn[:], whh_t[2][:], h_prev, start=True, stop=True)

                r = gates.tile([H, B], BF, tag="r")
                nc.scalar.activation(r[:], ps_r[:], AF.Sigmoid, bias=b_r)
                z = gates.tile([H, B], BF, tag="z")
                nc.scalar.activation(z[:], ps_z[:], AF.Sigmoid, bias=b_z)
                # narg = ps_in + (ps_hn + b_hn) * r
                tmp = work.tile([H, B], F32, tag="tmp")
                nc.vector.scalar_tensor_tensor(tmp[:], ps_hn[:], b_hn, r[:], OP.add, OP.mult)
                narg = work.tile([H, B], F32, tag="narg")
                nc.vector.tensor_add(narg[:], ps_in[:], tmp[:])
                n = gates.tile([H, B], BF, tag="n")
                nc.scalar.activation(n[:], narg[:], AF.Tanh, bias=b_in)
                # h' = n + z*(h - n)
                d = work.tile([H, B], BF, tag="d")
                nc.vector.tensor_sub(d[:], h_prev, n[:])
                zd = work.tile([H, B], BF, tag="zd")
                nc.vector.tensor_mul(zd[:], z[:], d[:])
                h_new = keysT[:, t * B:(t + 1) * B]
                nc.vector.tensor_add(h_new, n[:], zd[:])
                h_prev = h_new

        histT32 = cp.tile([D, B], F32)
        nc.vector.tensor_mul(histT32[:], hist_acc[:], invlen_bc[:])

        if debug:
            for j in range(25):
                seg = slice(j * 1024, (j + 1) * 1024)
                tmpd = work.tile([D, 1024], F32, tag="dbgk")
                nc.vector.tensor_copy(tmpd[:], keysT[:, seg])
                nc.sync.dma_start(dbg["keys"][:, seg], tmpd[:])

        # ================ P2: attention MLP ================
        if nphases < 2:
            stub = cp.tile([1, B], F32)
            nc.vector.tensor_copy(stub[:], keysT[0:1, 0:B])
            nc.sync.dma_start(out[:], stub[:])
            return
        ptBIG = big.tile([D, T * B], BF, tag="big2")
        kv = keysT[:].rearrange("p (t b) -> p t b", t=T)
        pv = ptBIG[:].rearrange("p (t b) -> p t b", t=T)

        with tc.tile_pool(name="att_ps", bufs=2, space="PSUM") as aps, \
             tc.tile_pool(name="attw", bufs=3) as aw:
            # pT = q * keys (t-major contiguous tiles of 2 t-steps)
            qbc = qT_t[:][:, None, :].broadcast_to([D, 2, B])
            for j in range(T // 2):
                ks = kv[:, 2 * j:2 * j + 2, :]
                ps = pv[:, 2 * j:2 * j + 2, :]
                nc.vector.tensor_mul(ps, ks, qbc)

            # attention MLP over b-major tiles
            for j in range(NT_ATT):
                bs = slice(j * BG, (j + 1) * BG)
                k_j = kv[:, :, bs].transpose([0, 2, 1])          # [D, BG, T]
                p_j = pv[:, :, bs].transpose([0, 2, 1])
                q_j = qT_t[:, bs, None].broadcast_to([D, BG, T])
                ps1 = aps.tile([80, BG * T], F32, tag="a1")
                o1 = ps1[:].rearrange("p (b t) -> p b t", b=BG)
                nc.tensor.matmul(o1, w0kf_t[:], k_j, start=True, stop=False)
                nc.tensor.matmul(o1, w0qf_t[:], q_j, start=False, stop=False)
                nc.tensor.matmul(o1, w0p_t[:], p_j, start=False, stop=True)
                a1 = aw.tile([80, BG * T], BF, tag="a1s")
                nc.scalar.activation(a1[:], ps1[:], AF.Relu, bias=b0_t[:])
                ps2 = aps.tile([40, BG * T], F32, tag="a2")
                nc.tensor.matmul(ps2[:], w1_t[:], a1[:], start=True, stop=True)
                a2 = aw.tile([40, BG * T], BF, tag="a2s")
                nc.scalar.activation(a2[:], ps2[:], AF.Relu, bias=b1_t[:])
                ps3 = aps.tile([1, BG * T], F32, tag="a3")
                nc.tensor.matmul(ps3[:], w2_t[:], a2[:], start=True, stop=True)
                s3row = aw.tile([1, BG * T], F32, tag="s3row")
                nc.vector.tensor_copy(s3row[:], ps3[:])
                nc.sync.dma_start(scoresDR[j], s3row[:])

        if debug:
            nc.sync.dma_start(dbg["scores"][:], scoresDR[:])

        if nphases < 3:
            stub = cp.tile([1, B], F32)
            nc.sync.dma_start(stub[:], scoresDR[0, None, 0:B])
            nc.sync.dma_start(out[:], stub[:])
            return
        # ================ P3: softmax + pooled ================
        attn_bf = cp.tile([BH, 2 * T], BF)
        attnT_sb = cp.tile([T, B], BF)
        scv = scoresDR[:].rearrange("j (b t) -> (j b) t", b=BG)     # [256, 100]
        with tc.tile_pool(name="sm_ps", bufs=2, space="PSUM") as sps, \
             tc.tile_pool(name="smw", bufs=2) as smw:
            for c in range(2):
                sc = smw.tile([BH, T], F32, tag="sc")
                nc.sync.dma_start(sc[:], scv[c * BH:(c + 1) * BH, :])
                E = smw.tile([BH, T], F32, tag="E")
                nc.scalar.activation(E[:], sc[:], AF.Exp, bias=b2_t[:])
                nc.vector.tensor_scalar_max(E[:], E[:], 1.0)
                nc.vector.tensor_mul(E[:], E[:], fmask_t[:, c, :])
                den = smw.tile([BH, 1], F32, tag="den")
                nc.vector.tensor_reduce(den[:], E[:], AX.X, OP.add)
                rec = smw.tile([BH, 1], F32, tag="rec")
                nc.vector.reciprocal(rec[:], den[:])
                nc.vector.tensor_scalar_mul(attn_bf[:, c * T:(c + 1) * T], E[:], rec[:])
                if debug:
                    af = smw.tile([BH, T], F32, tag="af32")
                    nc.vector.tensor_copy(af[:], attn_bf[:, c * T:(c + 1) * T])
                    nc.sync.dma_start(dbg["attn"][c * BH:(c + 1) * BH, :], af[:])
                pst = sps.tile([T, BH], BF, tag="tr")
                nc.tensor.transpose(pst[:], attn_bf[:, c * T:(c + 1) * T], eye_t[:])
                nc.vector.tensor_copy(attnT_sb[:, c * BH:(c + 1) * BH], pst[:])
        nc.sync.dma_start(attnDR[:], attnT_sb[:])

        # broadcast attn rows -> abig; P = keys * attn_bc; reduce over t
        abig = big.tile([D, T * B], BF, tag="big2")   # reuses ptBIG slot
        for t in range(T):
            _bcast_row(nc, abig[:, t * B:(t + 1) * B], attnDR[t:t + 1, :])
        for j in range(T * B // 512):
            seg = slice(j * 512, (j + 1) * 512)
            nc.vector.tensor_mul(abig[:, seg], keysT[:, seg], abig[:, seg])
        pooledT = cp.tile([D, B], F32)
        av = abig[:].rearrange("p (t b) -> p t b", t=T)
        nc.vector.tensor_reduce(pooledT[:], av.transpose([0, 2, 1]), AX.X, OP.add)
        pooled_bf = cp.tile([D, B], BF)
        nc.vector.tensor_copy(pooled_bf[:], pooledT[:])
        nc.sync.dma_start(pooledDR[:], pooled_bf[:])
        if debug:
            nc.sync.dma_start(dbg["pooled"][:], pooledT[:])
            nc.sync.dma_start(dbg["hist"][:], histT32[:])

        if nphases < 4:
            stub = cp.tile([1, B], F32)
            nc.vector.tensor_copy(stub[:], pooledT[0:1, :])
            nc.sync.dma_start(out[:], stub[:])
            return
        # ================ P4: AUGRU ================
        attf_acc = cp.tile([D, B], F32)
        nc.gpsimd.memset(attf_acc[:], 0.0)
        abc_p = ctx.enter_context(tc.tile_pool(name="abc", bufs=6))
        selT_v = bview("selT")
        with tc.tile_pool(name="aug_ps", bufs=2, space="PSUM") as ups:
            h_prev = zeros_bf[:]
            for t in range(T):
                k_t = keysT[:, t * B:(t + 1) * B]
                abc = abc_p.tile([128, B], BF, tag="abc")
                _bcast_row(nc, abc[:], pooledDR[t:t + 1, :])
                selbc = abc_p.tile([128, B], BF, tag="selbc")
                _bcast_row(nc, selbc[:], selT_v[t:t + 1, :])

                ps_r = ups.tile([H, B], F32, tag="r")
                ps_u = ups.tile([H, B], F32, tag="u")
                ps_h = ups.tile([H, B], F32, tag="hh")
                nc.tensor.matmul(ps_r[:], wa_x_t[0][:], k_t, start=True, stop=False)
                nc.tensor.matmul(ps_r[:], wa_h_t[0][:], h_prev, start=False, stop=True)
                nc.tensor.matmul(ps_u[:], wa_x_t[1][:], k_t, start=True, stop=False)
                nc.tensor.matmul(ps_u[:], wa_h_t[1][:], h_prev, start=False, stop=True)

                r = gates.tile([H, B], BF, tag="ar")
                nc.scalar.activation(r[:], ps_r[:], AF.Sigmoid, bias=ba_t[0][:])
                u = gates.tile([H, B], BF, tag="au")
                nc.scalar.activation(u[:], ps_u[:], AF.Sigmoid, bias=ba_t[1][:])
                rh = gates.tile([H, B], BF, tag="rh")
                nc.vector.tensor_mul(rh[:], r[:], h_prev)
                nc.tensor.matmul(ps_h[:], wa_x_t[2][:], k_t, start=True, stop=False)
                nc.tensor.matmul(ps_h[:], wa_h_t[2][:], rh[:], start=False, stop=True)
                hh = gates.tile([H, B], BF, tag="hh")
                nc.scalar.activation(hh[:], ps_h[:], AF.Tanh, bias=ba_t[2][:])

                up = gates.tile([H, B], BF, tag="up")
                nc.vector.tensor_mul(up[:], abc[:], u[:])
                dd = work.tile([H, B], BF, tag="add")
                nc.vector.tensor_sub(dd[:], hh[:], h_prev)
                ud = work.tile([H, B], BF, tag="aud")
                nc.vector.tensor_mul(ud[:], up[:], dd[:])
                h_new_t = gates.tile([H, B], BF, tag="ah")
                nc.vector.tensor_add(h_new_t[:], h_prev, ud[:])
                # attf += h_new * selbc  (gpsimd, off the critical path)
                sp = work.tile([H, B], BF, tag="sp")
                nc.gpsimd.tensor_mul(sp[:], h_new_t[:], selbc[:])
                nc.gpsimd.tensor_add(attf_acc[:], attf_acc[:], sp[:])
                h_prev = h_new_t[:]

        if nphases < 5:
            stub = cp.tile([1, B], F32)
            nc.vector.tensor_copy(stub[:], attf_acc[0:1, :])
            nc.sync.dma_start(out[:], stub[:])
            return
        # ================ P5: predict head ================
        attf_bf = cp.tile([D, B], BF)
        nc.vector.tensor_copy(attf_bf[:], attf_acc[:])
        if debug:
            nc.sync.dma_start(dbg["attf"][:], attf_acc[:])
        m2_bf = cp.tile([D, B], BF)
        nc.vector.tensor_mul(m2_bf[:], qT_t[:], histT32[:])
        hist_bf = cp.tile([D, B], BF)
        nc.vector.tensor_copy(hist_bf[:], histT32[:])

        comb = [uT_t[:], qT_t[:], hist_bf[:], m2_bf[:], attf_bf[:]]
        with tc.tile_pool(name="ph_ps", bufs=2, space="PSUM") as pps, \
             tc.tile_pool(name="phw", bufs=2) as pw:
            s1a_ps = pps.tile([128, B], F32, tag="s1a")
            s1b_ps = pps.tile([72, B], F32, tag="s1b")
            for i, blk in enumerate("uqhma"):
                nc.tensor.matmul(s1a_ps[:], ph0_t[blk][0][:], comb[i],
                                 start=(i == 0), stop=(i == 4))
                nc.tensor.matmul(s1b_ps[:], ph0_t[blk][1][:], comb[i],
                                 start=(i == 0), stop=(i == 4))
            s1a = pw.tile([128, B], BF, tag="s1a")
            nc.scalar.activation(s1a[:], s1a_ps[:], AF.Sigmoid, bias=bph0a_t[:])
            s1b = pw.tile([72, B], BF, tag="s1b")
            nc.scalar.activation(s1b[:], s1b_ps[:], AF.Sigmoid, bias=bph0b_t[:])
            s2_ps = pps.tile([80, B], F32, tag="s2")
            nc.tensor.matmul(s2_ps[:], ph1a_t[:], s1a[:], start=True, stop=False)
            nc.tensor.matmul(s2_ps[:], ph1b_t[:], s1b[:], start=False, stop=True)
            s2 = pw.tile([80, B], BF, tag="s2s")
            nc.scalar.activation(s2[:], s2_ps[:], AF.Sigmoid, bias=bph1_t[:])
            s3_ps = pps.tile([1, B], F32, tag="s3")
            nc.tensor.matmul(s3_ps[:], ph2_t[:], s2[:], start=True, stop=True)
            s3 = pw.tile([1, B], F32, tag="s3s")
            nc.scalar.activation(s3[:], s3_ps[:], AF.Sigmoid, bias=bph2_t[0:1, :])
            nc.sync.dma_start(out[:], s3[:])

    with tile.TileContext(nc) as tc, ExitStack() as ctx:
        _body(tc, ctx)
    return _finish(nc)


def _finish(nc):
    if not nc.is_finalized():
        nc.finalize()
    return nc


def _pack(vals, pack, dtype):
    flat = [np.ascontiguousarray(vals[name], dtype).reshape(-1) for name, _ in pack]
    return np.concatenate(flat).reshape(1, -1)


def _prep_in_maps(inputs):
    f = np.float32
    x = np.asarray(inputs["item_historical_embedding"], f)
    q = np.asarray(inputs["item_embedding"], f)
    u = np.asarray(inputs["user_embedding"], f)
    mask = np.asarray(inputs["mask"])
    lens = np.asarray(inputs["sequential_length"])

    W = {}
    gih = np.asarray(inputs["gru_Wih"], f)     # (3H, D)
    ghh = np.asarray(inputs["gru_Whh"], f)
    for i, g in enumerate("rzn"):
        W[f"wih_{g}"] = gih[i * H:(i + 1) * H, :].T
        W[f"whh_{g}"] = ghh[i * H:(i + 1) * H, :].T
    W["bihc"] = np.asarray(inputs["gru_bih"], f).reshape(3, H).T
    W["bhhc"] = np.asarray(inputs["gru_bhh"], f).reshape(3, H).T
    for g, wn, bn in (("r", "aug_Wr", "aug_br"), ("u", "aug_Wu", "aug_bu"),
                      ("h", "aug_Wh", "aug_bh")):
        wa = np.asarray(inputs[wn], f)                                # (H, D+H)
        W[f"wa{g}_h"] = wa[:, :H].T
        W[f"wa{g}_x"] = wa[:, H:].T
        W[f"ba_{g}"] = np.asarray(inputs[bn], f).reshape(H, 1)
    a0 = np.asarray(inputs["att_W0"], f)                              # (80, 512)
    w0k, w0q, w0d, w0p = (a0[:, i * D:(i + 1) * D].T for i in range(4))
    W["w0kf"] = w0k + w0d
    W["w0qf"] = w0q - w0d
    W["w0p"] = w0p
    W["b0"] = np.asarray(inputs["att_b0"], f).reshape(80, 1)
    W["w1"] = np.asarray(inputs["att_W1"], f).T
    W["b1"] = np.asarray(inputs["att_b1"], f).reshape(40, 1)
    W["w2"] = np.asarray(inputs["att_W2"], f).T
    W["b2rep"] = np.full((128, 1), float(np.asarray(inputs["att_b2"], f).reshape(-1)[0]), f)
    p0 = np.asarray(inputs["ph_W0"], f)                               # (200, 640)
    for i, blk in enumerate("uqhma"):
        blkW = p0[:, i * D:(i + 1) * D]                               # (200, 128)
        W[f"ph0_{blk}_a"] = blkW[:128, :].T
        W[f"ph0_{blk}_b"] = blkW[128:, :].T
    bp0 = np.asarray(inputs["ph_b0"], f)
    W["bph0a"] = bp0[:128].reshape(128, 1)
    W["bph0b"] = bp0[128:].reshape(72, 1)
    p1 = np.asarray(inputs["ph_W1"], f)                               # (80, 200)
    W["ph1a"] = p1[:, :128].T
    W["ph1b"] = p1[:, 128:].T
    W["bph1"] = np.asarray(inputs["ph_b1"], f).reshape(80, 1)
    W["ph2"] = np.asarray(inputs["ph_W2"], f).T
    W["bph2"] = np.asarray(inputs["ph_b2"], f).reshape(1, 1)
    W["eye"] = np.eye(128, dtype=f)

    in_maps = []
    for s in range(NCORES):
        sl = slice(s * B, (s + 1) * B)
        xs = x[sl]                       # (B, T, D)
        ms = mask[sl]                    # (B, T) int32
        V = dict(W)
        xm = xs * ms[:, :, None]         # host-masked: t >= len rows are 0
        V["qT"] = q[sl].T
        V["uT"] = u[sl].T
        V["fmask3"] = ms.astype(f).reshape(2, BH, T).transpose(1, 0, 2)
        V["invlen_row"] = (1.0 / lens[sl].astype(f)).reshape(1, B)
        sel = np.zeros((T, B), f)
        sel[np.asarray(lens[sl], np.int64) - 1, np.arange(B)] = 1.0
        V["selT"] = sel
        m = {
            "xT": np.ascontiguousarray(xm.transpose(1, 2, 0)).astype(XNP),  # [T, D, B]
            "wbf": _pack(V, PACK_BF, bf16),
            "wf32": _pack(V, PACK_F32, f),
        }
        in_maps.append(m)
    return in_maps


def get_nc(debug=False, nphases=5):
    key = ("nc", debug, nphases)
    if key not in _CACHED:
        _CACHED[key] = build_nc(debug=debug, nphases=nphases)
    return _CACHED[key]


def run_on_hw(inputs, debug=False):
    nc = get_nc(debug=debug)
    in_maps = _prep_in_maps(inputs)
    return run_bass_kernel_spmd(nc, in_maps, list(range(NCORES)))


def kernel(**inputs) -> np.ndarray:
    r = run_on_hw(inputs, debug=False)
    outs = [r.results[i]["out"].reshape(B) for i in range(NCORES)]
    return np.concatenate(outs).astype(np.float32)
